# revision 1
# baseline (speedup 1.0000x reference)
"""Trainium2 Bass kernel for CapsNet dynamic routing (ClassCapsules).

Reference computation (B=256, R=1152, C=10, O=16, I=8, 3 routing iters):
    u_hat[b,r,c,o] = sum_i W[r,c,o,i] * x[b,r,i]
    b_ij = 0
    for it in 3:
        c_ij = softmax(b_ij, axis=1)                      # over c
        s = sum_r c_ij[r,c] * u_hat[b,r,c,o] + bias       # [B,C,O]
        v = squash(s)
        if it < 2:
            b_ij += mean_b sum_o u_hat[b,r,c,o] v[b,c,o]  # [R,C]
    return v[..., None]

u_hat ([B,R,C,O] = 189MB fp32) is never materialized.  Both routing
contractions are re-associated through the factorization
    s[b,co]    = x~[b,(ri)] @ (c∘W~)[(ri),(co)]
    agree[r,c] = sum_{i,o} W~[(ri),(co)] * G[(ri),(co)],
                 G = (1/B) x~^T v
with x~ = x viewed as [B, R*I] and W~ = W viewed as [R*I, C*O].

Distribution: R is sharded 8 ways (144 r's per core).  Per iteration the
partial s ([256,160] fp32, 160KB) is summed across cores with one
AllReduce; the last iteration uses a ReduceScatter instead and each core
squashes + outputs its own 32-batch shard.  agree/b_ij/c_ij are fully
local to each core's r-shard.  An optional dependency-free warm-up
AllReduce at kernel start absorbs cross-core launch skew while inputs
load.
"""

import os
import sys
import types

sys.path.insert(0, "/opt/trn_rl_repo")

# Shim antenv.axon_hooks (absent on this image) so BASS_TRACE=1 profiling
# works through run_bass_kernel_spmd's axon path.  Harmless when unused.
try:
    import antenv.axon_hooks  # noqa: F401
except ImportError:
    try:
        _hooks = types.ModuleType("antenv.axon_hooks")
        _hooks._hook = None
        _hooks.set_axon_ntff_profile_hook = lambda h: setattr(_hooks, "_hook", h)
        _hooks.get_axon_ntff_profile_hook = lambda: _hooks._hook
        sys.modules["antenv.axon_hooks"] = _hooks
        import antenv
        antenv.axon_hooks = _hooks
        from trn_agent_boot.trn_boot import _ntff_profile_via_ctypes
        _hooks.set_axon_ntff_profile_hook(
            _ntff_profile_via_ctypes("/opt/axon/libaxon_pjrt.so")
        )
    except Exception:
        pass

import numpy as np

import concourse.bacc as bacc
import concourse.bass as bass
import concourse.tile as tile
from concourse import mybir
import concourse.bass_utils as _bass_utils
from concourse.bass_utils import run_bass_kernel_spmd

if os.environ.get("BASS_TRACE"):
    _bass_utils.upload_artifacts = lambda tmpdir: ""  # no bucket access here

LAST_RESULT = None

F32 = mybir.dt.float32
F16 = mybir.dt.float16
ALU = mybir.AluOpType
ACT = mybir.ActivationFunctionType

B, R, C, O, I = 256, 1152, 10, 16, 8
CO = C * O                      # 160
N_CORES = 8
R_LOC = R // N_CORES            # 144
RI_LOC = R_LOC * I              # 1152
NG = RI_LOC // 128              # 9 groups of 128 (r,i) rows
NB = B // 128                   # 2 batch partition chunks
B_SHARD = B // N_CORES          # 32 batches output per core
ITERS = 3
RPG = 128 // I                  # 16 r's per group

WARM_AR = os.environ.get("K_WARM_AR", "1") == "1"
CC_F16 = os.environ.get("K_CC_F16", "0") == "1"
CC_DT = F16 if CC_F16 else F32


def _squash(nc, eps_sb, t, n_part, nb, pool):
    """v = t * n2/((1+n2)*sqrt(n2+eps)); t: [n_part, nb, CO], reduce over o."""
    nc_ = nb * C
    tf = t.rearrange("p nb co -> p (nb co)")
    sq = pool.tile([n_part, nb * CO], F32, tag="sq")
    nc.vector.tensor_mul(sq, tf, tf)
    n2 = pool.tile([n_part, nc_], F32, tag="n2")
    nc.vector.reduce_sum(
        n2, sq.rearrange("p (nb c o) -> p nb c o", nb=nb, c=C),
        axis=mybir.AxisListType.X,
    )
    rt = pool.tile([n_part, nc_], F32, tag="rt")
    nc.scalar.activation(rt, n2, ACT.Sqrt, bias=eps_sb[:n_part])
    n2p1 = pool.tile([n_part, nc_], F32, tag="n2p1")
    nc.vector.tensor_scalar_add(n2p1, n2, 1.0)
    den = pool.tile([n_part, nc_], F32, tag="den")
    nc.vector.tensor_mul(den, n2p1, rt)
    rec = pool.tile([n_part, nc_], F32, tag="rec")
    nc.vector.reciprocal(rec, den)
    fac = pool.tile([n_part, nc_], F32, tag="fac")
    nc.vector.tensor_mul(fac, n2, rec)
    v = pool.tile([n_part, nb, CO], F32, tag="v")
    fac_b = fac.rearrange(
        "p (nb c one) -> p nb c one", nb=nb, c=C
    ).broadcast_to([n_part, nb, C, O])
    nc.vector.tensor_tensor(
        out=v.rearrange("p nb (c o) -> p nb c o", c=C),
        in0=t.rearrange("p nb (c o) -> p nb c o", c=C),
        in1=fac_b,
        op=ALU.mult,
    )
    return v


def build():
    nc = bacc.Bacc("TRN2", target_bir_lowering=False, debug=False,
                   num_devices=N_CORES)

    xt_d = nc.dram_tensor("xt", [RI_LOC, B], F32, kind="ExternalInput")
    xb_d = nc.dram_tensor("xb", [B, RI_LOC], F32, kind="ExternalInput")
    wg_d = nc.dram_tensor("wg", [RI_LOC, CO], F32, kind="ExternalInput")
    bias_d = nc.dram_tensor("biasf", [CO], F32, kind="ExternalInput")
    sel_d = nc.dram_tensor("sel", [128, RPG], F32, kind="ExternalInput")
    selT_d = nc.dram_tensor("selT", [RPG, 128], F32, kind="ExternalInput")
    y_d = nc.dram_tensor("y", [B_SHARD, CO], F32, kind="ExternalOutput")

    rg = [list(range(N_CORES))]

    with tile.TileContext(nc) as tc:
        with (
            tc.tile_pool(name="singles", bufs=1) as singles,
            tc.tile_pool(name="cw_pool", bufs=2) as cw_pool,
            tc.tile_pool(name="work", bufs=2) as work,
            tc.tile_pool(name="small", bufs=3) as small,
            tc.tile_pool(name="psum_s", bufs=1, space="PSUM") as psum_s,
            tc.tile_pool(name="psum_g", bufs=2, space="PSUM") as psum_g,
            tc.tile_pool(name="psum_misc", bufs=1, space="PSUM") as psum_misc,
            tc.tile_pool(name="dram", bufs=2, space="DRAM") as dram,
        ):
            if WARM_AR:
                warm_sb = singles.tile([1, 8], F32)
                nc.vector.memset(warm_sb, 0.0)
                warm_in = dram.tile([8], F32)
                warm_out = dram.tile([8], F32)
                nc.gpsimd.dma_start(out=warm_in[:], in_=warm_sb[0, :])
                nc.gpsimd.collective_compute(
                    "AllReduce", ALU.add, replica_groups=rg,
                    ins=[warm_in[:]], outs=[warm_out[:]],
                )

            # ---- load inputs ----
            XT = singles.tile([128, NG, B], F32)       # x~ [(ri),b] chunked
            nc.sync.dma_start(
                out=XT, in_=xt_d.ap().rearrange("(g p) b -> p g b", p=128)
            )
            XB = []                                    # x [b,(ri)] 2 p-chunks
            for kb in range(NB):
                t = singles.tile([128, RI_LOC], F32, tag=f"xb{kb}",
                                 name=f"xb_sb{kb}")
                nc.sync.dma_start(out=t, in_=xb_d[kb * 128:(kb + 1) * 128, :])
                XB.append(t)
            WG = singles.tile([128, NG, CO], F32)      # W~ [(ri),(co)] chunked
            nc.sync.dma_start(
                out=WG, in_=wg_d.ap().rearrange("(g p) n -> p g n", p=128)
            )
            biasb = singles.tile([128, CO], F32)
            nc.sync.dma_start(
                out=biasb,
                in_=bass.AP(tensor=bias_d, offset=0, ap=[[0, 128], [1, CO]]),
            )
            sel_sb = singles.tile([128, RPG], F32)
            nc.sync.dma_start(out=sel_sb, in_=sel_d[:, :])
            selT_sb = singles.tile([RPG, 128], F32)
            nc.sync.dma_start(out=selT_sb, in_=selT_d[:, :])

            eps_sb = singles.tile([128, 1], F32)
            nc.vector.memset(eps_sb, 1e-8)

            esr = None   # [16, 99]: exp(b_ij) ++ 1/sum_c exp(b_ij)

            for it in range(ITERS):
                # ---- CW = c∘W~ (it>0); it=0 uses uniform c=0.1 folded later
                if it == 0:
                    CW = WG
                else:
                    # cp_sb[:, :90] = e broadcast over i; [:, 90:99] = rec b.
                    cp_ps = psum_misc.tile([128, NG * C + NG], F32, tag="cp",
                                           name=f"cp_ps_{it}")
                    nc.tensor.matmul(cp_ps, selT_sb, esr, start=True, stop=True)
                    cp_sb = small.tile([128, NG * C + NG], F32, tag="cpart",
                                       name=f"cp_sb_{it}")
                    nc.scalar.copy(cp_sb, cp_ps)
                    CW = cw_pool.tile([128, NG, CO], F32, tag="cw",
                                      name=f"cw_{it}")
                    NGP = 6          # groups on DVE via stt; rest on gpsimd
                    cn = small.tile([128, (NG - NGP) * C], F32, tag="cn",
                                    name=f"cn_{it}")
                    rec_b3 = cp_sb[:, NG * C + NGP:NG * C + NG].rearrange(
                        "p (g one) -> p g one", one=1
                    ).broadcast_to([128, NG - NGP, C])
                    nc.vector.tensor_tensor(
                        out=cn.rearrange("p (g c) -> p g c", g=NG - NGP),
                        in0=cp_sb[:, NGP * C:NG * C].rearrange(
                            "p (g c) -> p g c", g=NG - NGP),
                        in1=rec_b3, op=ALU.mult,
                    )
                    for g in range(NG):
                        if g < NGP:
                            e_b = cp_sb[:, g * C:(g + 1) * C].rearrange(
                                "p (c one) -> p c one", one=1
                            ).broadcast_to([128, C, O])
                            nc.vector.scalar_tensor_tensor(
                                out=CW[:, g, :].rearrange(
                                    "p (c o) -> p c o", c=C),
                                in0=WG[:, g, :].rearrange(
                                    "p (c o) -> p c o", c=C),
                                scalar=cp_sb[:, NG * C + g:NG * C + g + 1],
                                in1=e_b,
                                op0=ALU.mult, op1=ALU.mult,
                            )
                        else:
                            c_b = cn[:, (g - NGP) * C:(g - NGP + 1) * C
                                     ].rearrange(
                                "p (c one) -> p c one", one=1
                            ).broadcast_to([128, C, O])
                            nc.gpsimd.tensor_tensor(
                                out=CW[:, g, :].rearrange(
                                    "p (c o) -> p c o", c=C),
                                in0=WG[:, g, :].rearrange(
                                    "p (c o) -> p c o", c=C),
                                in1=c_b, op=ALU.mult,
                            )

                # ---- s partial: [256,160] = x~^T @ CW, K = (ri) local ----
                s_ps = [psum_s.tile([128, CO], F32, tag=f"s{kb}",
                                    name=f"s_ps{kb}_{it}")
                        for kb in range(NB)]
                for kb in range(NB):
                    for g in range(NG):
                        nc.tensor.matmul(
                            s_ps[kb],
                            XT[:, g, kb * 128:(kb + 1) * 128],
                            CW[:, g, :],
                            start=(g == 0),
                            stop=(g == NG - 1),
                        )

                cc_in = dram.tile([NB, 128, CO], CC_DT, tag="cc_in",
                                  name=f"cc_in_{it}")
                for kb in range(NB):
                    s_stage = work.tile([128, CO], CC_DT, tag=f"sstage{kb}",
                                        name=f"s_stage{kb}_{it}")
                    nc.scalar.copy(s_stage, s_ps[kb])
                    nc.sync.dma_start(out=cc_in[kb, :, :], in_=s_stage)
                    if kb == NB - 1:
                        dsq = small.tile([1, 1], F32, tag="dsq",
                                         name=f"dsq_{it}")
                        nc.scalar.activation(dsq, s_stage[:1, :1], ACT.Sqrt,
                                             bias=eps_sb[:1], scale=0.0)

                if it < ITERS - 1:
                    # ---- AllReduce s; every core squashes the full batch
                    cc_out = dram.tile([NB, 128, CO], CC_DT, tag="cc_out",
                                       name=f"cc_out_{it}")
                    nc.gpsimd.collective_compute(
                        "AllReduce", ALU.add, replica_groups=rg,
                        ins=[cc_in.opt()], outs=[cc_out.opt()],
                    )
                    s_sb = work.tile([128, NB, CO], CC_DT, tag="ssb",
                                     name=f"s_sb_{it}")
                    for kb in range(NB):
                        nc.sync.dma_start(
                            out=s_sb[:, kb, :], in_=cc_out[kb, :, :]
                        )
                    warm_ps = psum_misc.tile([RPG, 512], F32, tag="warmps",
                                             name=f"warm_ps_{it}")
                    warm_rhs = XT[:, 0, :]            # [128, 256] static
                    for wi in range(14):
                        nc.tensor.matmul(
                            warm_ps[:, :B], sel_sb, warm_rhs,
                            start=(wi == 0), stop=True,
                            skip_group_check=True,
                        )
                    t = work.tile([128, NB, CO], F32, tag="t",
                                  name=f"t_{it}")
                    bias_b = biasb.rearrange(
                        "p (one co) -> p one co", one=1
                    ).broadcast_to([128, NB, CO])
                    nc.vector.scalar_tensor_tensor(
                        out=t, in0=s_sb,
                        scalar=(0.1 if it == 0 else 1.0),
                        in1=bias_b, op0=ALU.mult, op1=ALU.add,
                    )
                    v_sb = _squash(nc, eps_sb, t, 128, NB, work)

                    # ---- G = (1/B) x~^T v ; agree = sum_io W∘G ----
                    Q_all = small.tile([128, NG * C], F32, tag="qall",
                                       name=f"qall_{it}")
                    p9 = work.tile([128, NG, CO], F32, tag="p9",
                                   name=f"p9_{it}")
                    for g in range(NG):
                        g_ps = psum_g.tile([128, CO], F32, tag="gps",
                                           name=f"g_ps_{it}_{g}")
                        for kb in range(NB):
                            nc.tensor.matmul(
                                g_ps,
                                XB[kb][:, g * 128:(g + 1) * 128],
                                v_sb[:, kb, :],
                                start=(kb == 0),
                                stop=(kb == NB - 1),
                            )
                        nc.vector.scalar_tensor_tensor(
                            out=p9[:, g, :], in0=g_ps, scalar=1.0 / B,
                            in1=WG[:, g, :], op0=ALU.mult, op1=ALU.mult,
                        )
                        if g == 3 or g == 7 or g == 8:
                            lo = 0 if g == 3 else (4 if g == 7 else 8)
                            nc.vector.reduce_sum(
                                Q_all[:, lo * C:(g + 1) * C],
                                p9[:, lo:g + 1, :].rearrange(
                                    "p g (c o) -> p (g c) o", c=C),
                                axis=mybir.AxisListType.X,
                            )
                    agree_ps = psum_misc.tile([RPG, NG * C], F32, tag="agree",
                                              name=f"agree_{it}")
                    nc.tensor.matmul(agree_ps, sel_sb, Q_all,
                                     start=True, stop=True)

                    # ---- exp(b_ij) updated multiplicatively:
                    # exp(b_prev + agree) = exp(b_prev) * exp(agree) ----
                    esr_prev = esr
                    esr = small.tile([RPG, NG * C + NG], F32, tag="esr",
                                     name=f"esr_{it}")
                    if it == 0:
                        nc.scalar.activation(esr[:, :NG * C], agree_ps, ACT.Exp)
                    else:
                        eexp = small.tile([RPG, NG * C], F32, tag="eexp",
                                          name=f"eexp_{it}")
                        nc.scalar.activation(eexp, agree_ps, ACT.Exp)
                        nc.vector.tensor_mul(
                            esr[:, :NG * C], esr_prev[:, :NG * C], eexp
                        )
                    den = small.tile([RPG, NG], F32, tag="sden",
                                     name=f"den_{it}")
                    nc.vector.reduce_sum(
                        den,
                        esr[:, :NG * C].rearrange("p (g c) -> p g c", g=NG),
                        axis=mybir.AxisListType.X,
                    )
                    nc.vector.reciprocal(esr[:, NG * C:], den)
                else:
                    # ---- final iter: ReduceScatter; squash own b-shard ----
                    rs_out = dram.tile([B_SHARD * CO], CC_DT, tag="rs_out")
                    nc.gpsimd.collective_compute(
                        "ReduceScatter", ALU.add, replica_groups=rg,
                        ins=[cc_in.opt()], outs=[rs_out[:]],
                    )
                    s_sb = work.tile([B_SHARD, 1, CO], CC_DT, tag="fs")
                    nc.sync.dma_start(
                        out=s_sb,
                        in_=rs_out.rearrange("(p one n) -> p one n",
                                             n=CO, one=1),
                    )
                    t = work.tile([B_SHARD, 1, CO], F32, tag="ft")
                    bias_b1 = biasb[:B_SHARD, :].rearrange(
                        "p (one co) -> p one co", one=1
                    )
                    nc.vector.scalar_tensor_tensor(
                        out=t, in0=s_sb, scalar=1.0,
                        in1=bias_b1, op0=ALU.mult, op1=ALU.add,
                    )
                    v = _squash(nc, eps_sb, t, B_SHARD, 1, work)
                    nc.sync.dma_start(
                        out=y_d[:, :], in_=v.rearrange("p one co -> p (one co)")
                    )

    nc.compile()
    return nc


_NC = None


def kernel(x: np.ndarray, W: np.ndarray, bias: np.ndarray) -> np.ndarray:
    global _NC
    if _NC is None:
        _NC = build()

    x = np.ascontiguousarray(x, dtype=np.float32)
    W = np.ascontiguousarray(W, dtype=np.float32)
    bias = np.ascontiguousarray(bias, dtype=np.float32)

    biasf = bias.reshape(CO)
    sel = np.zeros((128, RPG), dtype=np.float32)
    sel[np.arange(128), np.arange(128) // I] = 1.0
    selT = np.ascontiguousarray(sel.T)

    in_maps = []
    for k in range(N_CORES):
        r0, r1 = k * R_LOC, (k + 1) * R_LOC
        xk = x[:, r0:r1, :].reshape(B, RI_LOC)          # [B,(r,i)]
        wk = W[r0:r1].transpose(0, 3, 1, 2).reshape(RI_LOC, CO)  # [(r,i),(c,o)]
        in_maps.append({
            "xt": np.ascontiguousarray(xk.T),
            "xb": np.ascontiguousarray(xk),
            "wg": np.ascontiguousarray(wk),
            "biasf": biasf,
            "sel": sel,
            "selT": selT,
        })

    global LAST_RESULT
    res = run_bass_kernel_spmd(
        _NC, in_maps, list(range(N_CORES)),
        trace=bool(os.environ.get("BASS_TRACE")),
    )
    LAST_RESULT = res
    v = np.concatenate([res.results[k]["y"] for k in range(N_CORES)], axis=0)
    return v.reshape(B, C, O)[..., None].astype(np.float32)



# revision 2
# speedup vs baseline: 1.0335x; 1.0335x over previous
"""Trainium2 Bass kernel for CapsNet dynamic routing (ClassCapsules).

Reference computation (B=256, R=1152, C=10, O=16, I=8, 3 routing iters):
    u_hat[b,r,c,o] = sum_i W[r,c,o,i] * x[b,r,i]
    b_ij = 0
    for it in 3:
        c_ij = softmax(b_ij, axis=1)                      # over c
        s = sum_r c_ij[r,c] * u_hat[b,r,c,o] + bias       # [B,C,O]
        v = squash(s)
        if it < 2:
            b_ij += mean_b sum_o u_hat[b,r,c,o] v[b,c,o]  # [R,C]
    return v[..., None]

u_hat ([B,R,C,O] = 189MB fp32) is never materialized.  Both routing
contractions are re-associated through the factorization
    s[b,co]    = x~[b,(ri)] @ (c∘W~)[(ri),(co)]
    agree[r,c] = sum_{i,o} W~[(ri),(co)] * G[(ri),(co)],
                 G = (1/B) x~^T v
with x~ = x viewed as [B, R*I] and W~ = W viewed as [R*I, C*O].

Distribution: R is sharded 8 ways (144 r's per core).  Per iteration the
partial s ([256,160]) is summed across cores with one AllReduce; the last
iteration uses a ReduceScatter instead and each core squashes + outputs
its own shard of batches.  agree/b_ij/c_ij are fully local to each
core's r-shard.

v2 changes vs v1: fp16 matmul operands (x / W host-precast, c∘W and v
staged in fp16), fp16 collectives, partition-major collective buffer
(one DMA back in), fused (1+n2)*rt on DVE, scalar-engine activation
tables (Sqrt/Exp) prefetched with dummy ops off the critical path, p9
multiply/reduce split across DVE and GpSimd, no warm-up AllReduce.
"""

import os
import sys
import types

sys.path.insert(0, "/opt/trn_rl_repo")

# Shim antenv.axon_hooks (absent on this image) so BASS_TRACE=1 profiling
# works through run_bass_kernel_spmd's axon path.  Harmless when unused.
try:
    import antenv.axon_hooks  # noqa: F401
except ImportError:
    try:
        _hooks = types.ModuleType("antenv.axon_hooks")
        _hooks._hook = None
        _hooks.set_axon_ntff_profile_hook = lambda h: setattr(_hooks, "_hook", h)
        _hooks.get_axon_ntff_profile_hook = lambda: _hooks._hook
        sys.modules["antenv.axon_hooks"] = _hooks
        import antenv
        antenv.axon_hooks = _hooks
        from trn_agent_boot.trn_boot import _ntff_profile_via_ctypes
        _hooks.set_axon_ntff_profile_hook(
            _ntff_profile_via_ctypes("/opt/axon/libaxon_pjrt.so")
        )
    except Exception:
        pass

import numpy as np

import concourse.bacc as bacc
import concourse.bass as bass
import concourse.tile as tile
from concourse import mybir
import concourse.bass_utils as _bass_utils
from concourse.bass_utils import run_bass_kernel_spmd

if os.environ.get("BASS_TRACE"):
    _bass_utils.upload_artifacts = lambda tmpdir: ""  # no bucket access here

LAST_RESULT = None

F32 = mybir.dt.float32
F16 = mybir.dt.float16
ALU = mybir.AluOpType
ACT = mybir.ActivationFunctionType

B, R, C, O, I = 256, 1152, 10, 16, 8
CO = C * O                      # 160
N_CORES = 8
R_LOC = R // N_CORES            # 144
RI_LOC = R_LOC * I              # 1152
NG = RI_LOC // 128              # 9 groups of 128 (r,i) rows
NB = B // 128                   # 2 batch partition chunks
P_SHARD = 128 // N_CORES        # 16 partition rows per core in ReduceScatter
ITERS = 3
RPG = 128 // I                  # 16 r's per group

WARM_AR = os.environ.get("K_WARM_AR", "0") == "1"
WARM_MM = int(os.environ.get("K_WARM_MM", "24"))


def _squash(nc, eps_sb, t, n_part, nb, pool, out_dt=F16):
    """v = t * n2/((1+n2)*sqrt(n2+eps)); t: [n_part, nb, CO], reduce over o."""
    nc_ = nb * C
    tf = t.rearrange("p nb co -> p (nb co)")
    sq = pool.tile([n_part, nb * CO], F32, tag="sq")
    nc.vector.tensor_mul(sq, tf, tf)
    n2 = pool.tile([n_part, nc_], F32, tag="n2")
    nc.vector.reduce_sum(
        n2, sq.rearrange("p (nb c o) -> p nb c o", nb=nb, c=C),
        axis=mybir.AxisListType.X,
    )
    rt = pool.tile([n_part, nc_], F32, tag="rt")
    nc.scalar.activation(rt, n2, ACT.Sqrt, bias=eps_sb[:n_part])
    den = pool.tile([n_part, nc_], F32, tag="den")
    nc.vector.scalar_tensor_tensor(
        out=den, in0=n2, scalar=1.0, in1=rt, op0=ALU.add, op1=ALU.mult,
    )
    rec = pool.tile([n_part, nc_], F32, tag="rec")
    nc.vector.reciprocal(rec, den)
    fac = pool.tile([n_part, nc_], F32, tag="fac")
    nc.vector.tensor_mul(fac, n2, rec)
    v = pool.tile([n_part, nb, CO], out_dt, tag="v")
    fac_b = fac.rearrange(
        "p (nb c one) -> p nb c one", nb=nb, c=C
    ).broadcast_to([n_part, nb, C, O])
    nc.vector.tensor_tensor(
        out=v.rearrange("p nb (c o) -> p nb c o", c=C),
        in0=t.rearrange("p nb (c o) -> p nb c o", c=C),
        in1=fac_b,
        op=ALU.mult,
    )
    return v


def build():
    nc = bacc.Bacc("TRN2", target_bir_lowering=False, debug=False,
                   num_devices=N_CORES)

    xt_d = nc.dram_tensor("xt", [RI_LOC, B], F16, kind="ExternalInput")
    xb_d = nc.dram_tensor("xb", [B, RI_LOC], F16, kind="ExternalInput")
    wg_d = nc.dram_tensor("wg", [RI_LOC, CO], F16, kind="ExternalInput")
    bias_d = nc.dram_tensor("biasf", [CO], F32, kind="ExternalInput")
    sel_d = nc.dram_tensor("sel", [128, RPG], F32, kind="ExternalInput")
    selT_d = nc.dram_tensor("selT", [RPG, 128], F32, kind="ExternalInput")
    y_d = nc.dram_tensor("y", [P_SHARD, NB * CO], F32, kind="ExternalOutput")

    rg = [list(range(N_CORES))]

    with tile.TileContext(nc) as tc:
        with (
            tc.tile_pool(name="singles", bufs=1) as singles,
            tc.tile_pool(name="cw_pool", bufs=2) as cw_pool,
            tc.tile_pool(name="work", bufs=2) as work,
            tc.tile_pool(name="small", bufs=3) as small,
            tc.tile_pool(name="psum_s", bufs=1, space="PSUM") as psum_s,
            tc.tile_pool(name="psum_g", bufs=2, space="PSUM") as psum_g,
            tc.tile_pool(name="psum_misc", bufs=1, space="PSUM") as psum_misc,
            tc.tile_pool(name="dram", bufs=2, space="DRAM") as dram,
        ):
            if WARM_AR:
                warm_sb = singles.tile([1, 8], F32)
                nc.vector.memset(warm_sb, 0.0)
                warm_in = dram.tile([8], F32)
                warm_out = dram.tile([8], F32)
                nc.gpsimd.dma_start(out=warm_in[:], in_=warm_sb[0, :])
                nc.gpsimd.collective_compute(
                    "AllReduce", ALU.add, replica_groups=rg,
                    ins=[warm_in[:]], outs=[warm_out[:]],
                )

            # ---- load inputs ----
            XT = singles.tile([128, NG, B], F16)       # x~ [(ri),b] chunked
            nc.sync.dma_start(
                out=XT, in_=xt_d.ap().rearrange("(g p) b -> p g b", p=128)
            )
            XB = []                                    # x [b,(ri)] 2 p-chunks
            for kb in range(NB):
                t = singles.tile([128, RI_LOC], F16, tag=f"xb{kb}",
                                 name=f"xb_sb{kb}")
                nc.sync.dma_start(out=t, in_=xb_d[kb * 128:(kb + 1) * 128, :])
                XB.append(t)
            WG = singles.tile([128, NG, CO], F16)      # W~ [(ri),(co)] chunked
            nc.sync.dma_start(
                out=WG, in_=wg_d.ap().rearrange("(g p) n -> p g n", p=128)
            )
            biasb = singles.tile([128, CO], F32)
            nc.sync.dma_start(
                out=biasb,
                in_=bass.AP(tensor=bias_d, offset=0, ap=[[0, 128], [1, CO]]),
            )
            sel_sb = singles.tile([128, RPG], F32)
            nc.sync.dma_start(out=sel_sb, in_=sel_d[:, :])
            selT_sb = singles.tile([RPG, 128], F32)
            nc.sync.dma_start(out=selT_sb, in_=selT_d[:, :])

            eps_sb = singles.tile([128, 1], F32)
            nc.vector.memset(eps_sb, 1e-8)
            junk = singles.tile([1, 1], F32)
            nc.vector.memset(junk, 1.0)
            # Prefetch the Sqrt activation table while inputs load.
            tl0 = singles.tile([1, 1], F32, tag="tl0")
            nc.scalar.activation(tl0, junk, ACT.Sqrt)

            esr = None   # [16, 99]: exp(b_ij) ++ 1/sum_c exp(b_ij)

            for it in range(ITERS):
                # ---- CW = c∘W~ (it>0); it=0 uses uniform c=0.1 folded later
                if it == 0:
                    CW = WG
                else:
                    # cp_sb[:, :90] = e broadcast over i; [:, 90:99] = rec b.
                    cp_ps = psum_misc.tile([128, NG * C + NG], F32, tag="cp",
                                           name=f"cp_ps_{it}")
                    nc.tensor.matmul(cp_ps, selT_sb, esr, start=True, stop=True)
                    cp_sb = small.tile([128, NG * C + NG], F32, tag="cpart",
                                       name=f"cp_sb_{it}")
                    nc.vector.tensor_copy(cp_sb, cp_ps)
                    CW = cw_pool.tile([128, NG, CO], F16, tag="cw",
                                      name=f"cw_{it}")
                    NGP = 6          # groups on DVE via stt; rest on gpsimd
                    cn = small.tile([128, (NG - NGP) * C], F32, tag="cn",
                                    name=f"cn_{it}")
                    rec_b3 = cp_sb[:, NG * C + NGP:NG * C + NG].rearrange(
                        "p (g one) -> p g one", one=1
                    ).broadcast_to([128, NG - NGP, C])
                    nc.vector.tensor_tensor(
                        out=cn.rearrange("p (g c) -> p g c", g=NG - NGP),
                        in0=cp_sb[:, NGP * C:NG * C].rearrange(
                            "p (g c) -> p g c", g=NG - NGP),
                        in1=rec_b3, op=ALU.mult,
                    )
                    for g in range(NG):
                        if g < NGP:
                            e_b = cp_sb[:, g * C:(g + 1) * C].rearrange(
                                "p (c one) -> p c one", one=1
                            ).broadcast_to([128, C, O])
                            nc.vector.scalar_tensor_tensor(
                                out=CW[:, g, :].rearrange(
                                    "p (c o) -> p c o", c=C),
                                in0=WG[:, g, :].rearrange(
                                    "p (c o) -> p c o", c=C),
                                scalar=cp_sb[:, NG * C + g:NG * C + g + 1],
                                in1=e_b,
                                op0=ALU.mult, op1=ALU.mult,
                            )
                        else:
                            c_b = cn[:, (g - NGP) * C:(g - NGP + 1) * C
                                     ].rearrange(
                                "p (c one) -> p c one", one=1
                            ).broadcast_to([128, C, O])
                            nc.gpsimd.tensor_tensor(
                                out=CW[:, g, :].rearrange(
                                    "p (c o) -> p c o", c=C),
                                in0=WG[:, g, :].rearrange(
                                    "p (c o) -> p c o", c=C),
                                in1=c_b, op=ALU.mult,
                            )

                # ---- s partial: [256,160] = x~^T @ CW, K = (ri) local ----
                s_ps = [psum_s.tile([128, CO], F32, tag=f"s{kb}",
                                    name=f"s_ps{kb}_{it}")
                        for kb in range(NB)]
                for kb in range(NB):
                    for g in range(NG):
                        nc.tensor.matmul(
                            s_ps[kb],
                            XT[:, g, kb * 128:(kb + 1) * 128],
                            CW[:, g, :],
                            start=(g == 0),
                            stop=(g == NG - 1),
                        )

                # Partition-major collective buffer: row p holds s for
                # batches (p, 128+p) at columns [0:CO] / [CO:2*CO].
                cc_in = dram.tile([128, NB * CO], F16, tag="cc_in",
                                  name=f"cc_in_{it}")
                for kb in range(NB):
                    s_stage = work.tile([128, CO], F16, tag=f"sstage{kb}",
                                        name=f"s_stage{kb}_{it}")
                    nc.scalar.copy(s_stage, s_ps[kb])
                    nc.sync.dma_start(
                        out=cc_in[:, kb * CO:(kb + 1) * CO], in_=s_stage
                    )

                if it < ITERS - 1:
                    # ---- AllReduce s; every core squashes the full batch
                    cc_out = dram.tile([128, NB * CO], F16, tag="cc_out",
                                       name=f"cc_out_{it}")
                    nc.gpsimd.collective_compute(
                        "AllReduce", ALU.add, replica_groups=rg,
                        ins=[cc_in.opt()], outs=[cc_out.opt()],
                    )
                    s_sb = work.tile([128, NB, CO], F16, tag="ssb",
                                     name=f"s_sb_{it}")
                    nc.sync.dma_start(
                        out=s_sb.rearrange("p nb co -> p (nb co)"),
                        in_=cc_out[:, :],
                    )
                    # Keep the PE HAM busy during the AllReduce so the
                    # G matmuls start at full clock.
                    warm_ps = psum_misc.tile([RPG, 512], F32, tag="warmps",
                                             name=f"warm_ps_{it}")
                    warm_rhs = XT[:, 0, :]            # [128, 256] static
                    for wi in range(WARM_MM):
                        nc.tensor.matmul(
                            warm_ps[:, :B], sel_sb, warm_rhs,
                            start=(wi == 0), stop=True,
                            skip_group_check=True,
                        )
                    t = work.tile([128, NB, CO], F32, tag="t",
                                  name=f"t_{it}")
                    bias_b = biasb.rearrange(
                        "p (one co) -> p one co", one=1
                    ).broadcast_to([128, NB, CO])
                    nc.vector.scalar_tensor_tensor(
                        out=t, in0=s_sb,
                        scalar=(0.1 if it == 0 else 1.0),
                        in1=bias_b, op0=ALU.mult, op1=ALU.add,
                    )
                    v_sb = _squash(nc, eps_sb, t, 128, NB, work, out_dt=F16)
                    # Prefetch the Exp table (runs during the G matmuls).
                    tlE = small.tile([1, 1], F32, tag=f"tlE",
                                     name=f"tlE_{it}")
                    nc.scalar.activation(tlE, junk, ACT.Exp)

                    # ---- G = (1/B) x~^T v ; agree = sum_io W∘G ----
                    Q_all = small.tile([128, NG * C], F32, tag="qall",
                                       name=f"qall_{it}")
                    p9 = work.tile([128, NG, CO], F32, tag="p9",
                                   name=f"p9_{it}")
                    for g in range(NG):
                        g_ps = psum_g.tile([128, CO], F32, tag="gps",
                                           name=f"g_ps_{it}_{g}")
                        for kb in range(NB):
                            nc.tensor.matmul(
                                g_ps,
                                XB[kb][:, g * 128:(g + 1) * 128],
                                v_sb[:, kb, :],
                                start=(kb == 0),
                                stop=(kb == NB - 1),
                            )
                        eng = nc.vector if g % 2 == 0 else nc.gpsimd
                        eng.scalar_tensor_tensor(
                            out=p9[:, g, :], in0=g_ps, scalar=1.0 / B,
                            in1=WG[:, g, :], op0=ALU.mult, op1=ALU.mult,
                        )
                        if g == 3 or g == 7 or g == 8:
                            lo = 0 if g == 3 else (4 if g == 7 else 8)
                            red = nc.vector if g == 3 else (
                                nc.gpsimd if g == 7 else nc.vector)
                            red.tensor_reduce(
                                op=ALU.add,
                                out=Q_all[:, lo * C:(g + 1) * C],
                                in_=p9[:, lo:g + 1, :].rearrange(
                                    "p g (c o) -> p (g c) o", c=C),
                                axis=mybir.AxisListType.X,
                            )
                    agree_ps = psum_misc.tile([RPG, NG * C], F32, tag="agree",
                                              name=f"agree_{it}")
                    nc.tensor.matmul(agree_ps, sel_sb, Q_all,
                                     start=True, stop=True)

                    # ---- exp(b_ij) updated multiplicatively:
                    # exp(b_prev + agree) = exp(b_prev) * exp(agree) ----
                    esr_prev = esr
                    esr = small.tile([RPG, NG * C + NG], F32, tag="esr",
                                     name=f"esr_{it}")
                    if it == 0:
                        nc.scalar.activation(esr[:, :NG * C], agree_ps, ACT.Exp)
                    else:
                        eexp = small.tile([RPG, NG * C], F32, tag="eexp",
                                          name=f"eexp_{it}")
                        nc.scalar.activation(eexp, agree_ps, ACT.Exp)
                        nc.vector.tensor_mul(
                            esr[:, :NG * C], esr_prev[:, :NG * C], eexp
                        )
                    # Prefetch Sqrt for the next squash (runs during CW/s).
                    tlS = small.tile([1, 1], F32, tag="tlS",
                                     name=f"tlS_{it}")
                    nc.scalar.activation(tlS, junk, ACT.Sqrt)
                    den = small.tile([RPG, NG], F32, tag="sden",
                                     name=f"den_{it}")
                    nc.vector.reduce_sum(
                        den,
                        esr[:, :NG * C].rearrange("p (g c) -> p g c", g=NG),
                        axis=mybir.AxisListType.X,
                    )
                    nc.vector.reciprocal(esr[:, NG * C:], den)
                else:
                    # ---- final iter: ReduceScatter; squash own shard ----
                    # Shard k of the flat [128*NB*CO] buffer = partition
                    # rows [16k, 16k+16) = batches 16k+j and 128+16k+j.
                    rs_out = dram.tile([P_SHARD * NB * CO], F16, tag="rs_out")
                    nc.gpsimd.collective_compute(
                        "ReduceScatter", ALU.add, replica_groups=rg,
                        ins=[cc_in.opt()], outs=[rs_out[:]],
                    )
                    s_sb = work.tile([P_SHARD, NB, CO], F16, tag="fs")
                    nc.sync.dma_start(
                        out=s_sb,
                        in_=rs_out.rearrange("(p nb n) -> p nb n",
                                             n=CO, nb=NB),
                    )
                    t = work.tile([P_SHARD, NB, CO], F32, tag="ft")
                    bias_b1 = biasb[:P_SHARD, :].rearrange(
                        "p (one co) -> p one co", one=1
                    ).broadcast_to([P_SHARD, NB, CO])
                    nc.vector.scalar_tensor_tensor(
                        out=t, in0=s_sb, scalar=1.0,
                        in1=bias_b1, op0=ALU.mult, op1=ALU.add,
                    )
                    v = _squash(nc, eps_sb, t, P_SHARD, NB, work, out_dt=F32)
                    nc.sync.dma_start(
                        out=y_d[:, :], in_=v.rearrange("p nb co -> p (nb co)")
                    )

    nc.compile()
    return nc


_NC = None


def kernel(x: np.ndarray, W: np.ndarray, bias: np.ndarray) -> np.ndarray:
    global _NC
    if _NC is None:
        _NC = build()

    x = np.ascontiguousarray(x, dtype=np.float32)
    W = np.ascontiguousarray(W, dtype=np.float32)
    bias = np.ascontiguousarray(bias, dtype=np.float32)

    biasf = bias.reshape(CO)
    sel = np.zeros((128, RPG), dtype=np.float32)
    sel[np.arange(128), np.arange(128) // I] = 1.0
    selT = np.ascontiguousarray(sel.T)

    in_maps = []
    for k in range(N_CORES):
        r0, r1 = k * R_LOC, (k + 1) * R_LOC
        xk = x[:, r0:r1, :].reshape(B, RI_LOC)          # [B,(r,i)]
        wk = W[r0:r1].transpose(0, 3, 1, 2).reshape(RI_LOC, CO)  # [(r,i),(c,o)]
        in_maps.append({
            "xt": np.ascontiguousarray(xk.T).astype(np.float16),
            "xb": np.ascontiguousarray(xk).astype(np.float16),
            "wg": np.ascontiguousarray(wk).astype(np.float16),
            "biasf": biasf,
            "sel": sel,
            "selT": selT,
        })

    global LAST_RESULT
    res = run_bass_kernel_spmd(
        _NC, in_maps, list(range(N_CORES)),
        trace=bool(os.environ.get("BASS_TRACE")),
    )
    LAST_RESULT = res
    # Reassemble: core k, row j, chunk kb  ->  batch kb*128 + 16*k + j.
    out = np.empty((B, CO), dtype=np.float32)
    for k in range(N_CORES):
        yk = res.results[k]["y"].reshape(P_SHARD, NB, CO)
        for kb in range(NB):
            out[kb * 128 + P_SHARD * k: kb * 128 + P_SHARD * (k + 1)] = \
                yk[:, kb, :]
    return out.reshape(B, C, O)[..., None].astype(np.float32)


# revision 6
# speedup vs baseline: 1.2236x; 1.1839x over previous
"""Trainium2 Bass kernel for CapsNet dynamic routing (ClassCapsules).

Reference computation (B=256, R=1152, C=10, O=16, I=8, 3 routing iters):
    u_hat[b,r,c,o] = sum_i W[r,c,o,i] * x[b,r,i]
    b_ij = 0
    for it in 3:
        c_ij = softmax(b_ij, axis=1)                      # over c
        s = sum_r c_ij[r,c] * u_hat[b,r,c,o] + bias       # [B,C,O]
        v = squash(s)
        if it < 2:
            b_ij += mean_b sum_o u_hat[b,r,c,o] v[b,c,o]  # [R,C]
    return v[..., None]

u_hat ([B,R,C,O] = 189MB fp32) is never materialized.  Both routing
contractions are re-associated through the factorization
    s[b,co]    = x~[b,(ri)] @ (c∘W~)[(ri),(co)]
    agree[r,c] = sum_{i,o} W~[(ri),(co)] * G[(ri),(co)],
                 G = (1/B) x~^T v
with x~ = x viewed as [B, R*I] and W~ = W viewed as [R*I, C*O].

Distribution: R is sharded 8 ways (144 r's per core) for iterations 1-2.
Iteration 0's c is UNIFORM (softmax of zeros), so s0 = 0.1*(x~ @ W~)
does not depend on c at all: every core redundantly computes the full
s0 from replicated fp16 copies of x~/W~ — the loads and the 144-matmul
accumulation hide inside the cross-core launch-skew window that the
first collective would have to absorb anyway.  This removes iteration
0's AllReduce (and its +-30us of barrier-serialized latency) from the
critical path.  Iteration 1 sums the partial s with one fp16 AllReduce
(which doubles as the rank-sync barrier); iteration 2 uses a fp16
ReduceScatter and each core squashes + outputs its own batch rows.
agree/b_ij/c_ij are local to each core's r-shard.

All matmul operands are fp16 (host-precast); accumulation fp32 in PSUM.
Scalar-engine activation tables (Sqrt/Exp) are prefetched with dummy
ops so table loads stay off the critical path.
"""

import os
import sys
import types

sys.path.insert(0, "/opt/trn_rl_repo")

# Shim antenv.axon_hooks (absent on this image) so BASS_TRACE=1 profiling
# works through run_bass_kernel_spmd's axon path.  Harmless when unused.
try:
    import antenv.axon_hooks  # noqa: F401
except ImportError:
    try:
        _hooks = types.ModuleType("antenv.axon_hooks")
        _hooks._hook = None
        _hooks.set_axon_ntff_profile_hook = lambda h: setattr(_hooks, "_hook", h)
        _hooks.get_axon_ntff_profile_hook = lambda: _hooks._hook
        sys.modules["antenv.axon_hooks"] = _hooks
        import antenv
        antenv.axon_hooks = _hooks
        from trn_agent_boot.trn_boot import _ntff_profile_via_ctypes
        _hooks.set_axon_ntff_profile_hook(
            _ntff_profile_via_ctypes("/opt/axon/libaxon_pjrt.so")
        )
    except Exception:
        pass

import numpy as np

import concourse.bacc as bacc
import concourse.bass as bass
import concourse.tile as tile
from concourse import mybir
import concourse.bass_utils as _bass_utils
from concourse.bass_utils import run_bass_kernel_spmd

if os.environ.get("BASS_TRACE"):
    _bass_utils.upload_artifacts = lambda tmpdir: ""  # no bucket access here

LAST_RESULT = None

F32 = mybir.dt.float32
F16 = mybir.dt.float16
ALU = mybir.AluOpType
ACT = mybir.ActivationFunctionType

B, R, C, O, I = 256, 1152, 10, 16, 8
CO = C * O                      # 160
N_CORES = 8
R_LOC = R // N_CORES            # 144
RI_LOC = R_LOC * I              # 1152
NG = RI_LOC // 128              # 9 groups of 128 (r,i) rows
NB = B // 128                   # 2 batch partition chunks
P_SHARD = 128 // N_CORES        # 16 partition rows per core in ReduceScatter
ITERS = 3
RPG = 128 // I                  # 16 r's per group

WARM_MM = int(os.environ.get("K_WARM_MM", "24"))


def _squash(nc, eps_sb, t, n_part, nb, pool, out_dt=F16):
    """v = t * n2/((1+n2)*sqrt(n2+eps)); t: [n_part, nb, CO], reduce over o."""
    nc_ = nb * C
    tf = t.rearrange("p nb co -> p (nb co)")
    sq = pool.tile([n_part, nb * CO], F32, tag="sq")
    nc.vector.tensor_mul(sq, tf, tf)
    n2 = pool.tile([n_part, nc_], F32, tag="n2")
    nc.vector.reduce_sum(
        n2, sq.rearrange("p (nb c o) -> p nb c o", nb=nb, c=C),
        axis=mybir.AxisListType.X,
    )
    rt = pool.tile([n_part, nc_], F32, tag="rt")
    nc.scalar.activation(rt, n2, ACT.Sqrt, bias=eps_sb[:n_part])
    den = pool.tile([n_part, nc_], F32, tag="den")
    nc.vector.scalar_tensor_tensor(
        out=den, in0=n2, scalar=1.0, in1=rt, op0=ALU.add, op1=ALU.mult,
    )
    rec = pool.tile([n_part, nc_], F32, tag="rec")
    nc.vector.reciprocal(rec, den)
    fac = pool.tile([n_part, nc_], F32, tag="fac")
    nc.vector.tensor_mul(fac, n2, rec)
    v = pool.tile([n_part, nb, CO], out_dt, tag="v")
    fac_b = fac.rearrange(
        "p (nb c one) -> p nb c one", nb=nb, c=C
    ).broadcast_to([n_part, nb, C, O])
    nc.vector.tensor_tensor(
        out=v.rearrange("p nb (c o) -> p nb c o", c=C),
        in0=t.rearrange("p nb (c o) -> p nb c o", c=C),
        in1=fac_b,
        op=ALU.mult,
    )
    return v


def build():
    nc = bacc.Bacc("TRN2", target_bir_lowering=False, debug=False,
                   num_devices=N_CORES)

    xtf_d = nc.dram_tensor("xtf", [R * I, B], F16, kind="ExternalInput")
    wgf_d = nc.dram_tensor("wgf", [R * I, CO], F16, kind="ExternalInput")
    xt_d = nc.dram_tensor("xt", [RI_LOC, B], F16, kind="ExternalInput")
    xb_d = nc.dram_tensor("xb", [B, RI_LOC], F16, kind="ExternalInput")
    wg_d = nc.dram_tensor("wg", [RI_LOC, CO], F16, kind="ExternalInput")
    bias_d = nc.dram_tensor("biasf", [CO], F32, kind="ExternalInput")
    sel_d = nc.dram_tensor("sel", [128, RPG], F32, kind="ExternalInput")
    selT_d = nc.dram_tensor("selT", [RPG, 128], F32, kind="ExternalInput")
    y_d = nc.dram_tensor("y", [P_SHARD, NB * CO], F32, kind="ExternalOutput")

    rg = [list(range(N_CORES))]

    with tile.TileContext(nc) as tc:
        with (
            tc.tile_pool(name="singles", bufs=1) as singles,
            tc.tile_pool(name="cw_pool", bufs=2) as cw_pool,
            tc.tile_pool(name="work", bufs=2) as work,
            tc.tile_pool(name="small", bufs=3) as small,
            tc.tile_pool(name="psum_s", bufs=1, space="PSUM") as psum_s,
            tc.tile_pool(name="psum_g", bufs=2, space="PSUM") as psum_g,
            tc.tile_pool(name="psum_misc", bufs=1, space="PSUM") as psum_misc,
            tc.tile_pool(name="dram", bufs=2, space="DRAM") as dram,
        ):
            # ---- load inputs: replicated full x~/W~ first (s0 consumes
            # them chunk by chunk), then the per-core shard tensors.
            XTF, WGF = [], []
            for cc in range(N_CORES):
                tx = singles.tile([128, NG, B], F16, tag=f"xtf{cc}",
                                  name=f"xtf_sb{cc}")
                nc.sync.dma_start(
                    out=tx,
                    in_=xtf_d[cc * RI_LOC:(cc + 1) * RI_LOC, :].rearrange(
                        "(g p) b -> p g b", p=128),
                )
                XTF.append(tx)
                tw = singles.tile([128, NG, CO], F16, tag=f"wgf{cc}",
                                  name=f"wgf_sb{cc}")
                nc.sync.dma_start(
                    out=tw,
                    in_=wgf_d[cc * RI_LOC:(cc + 1) * RI_LOC, :].rearrange(
                        "(g p) n -> p g n", p=128),
                )
                WGF.append(tw)

            XT = singles.tile([128, NG, B], F16)       # local x~ [(ri),b]
            nc.sync.dma_start(
                out=XT, in_=xt_d.ap().rearrange("(g p) b -> p g b", p=128)
            )
            XB = []                                    # x [b,(ri)] 2 p-chunks
            for kb in range(NB):
                t = singles.tile([128, RI_LOC], F16, tag=f"xb{kb}",
                                 name=f"xb_sb{kb}")
                nc.sync.dma_start(out=t, in_=xb_d[kb * 128:(kb + 1) * 128, :])
                XB.append(t)
            WG = singles.tile([128, NG, CO], F16)      # local W~ [(ri),(co)]
            nc.sync.dma_start(
                out=WG, in_=wg_d.ap().rearrange("(g p) n -> p g n", p=128)
            )
            biasb = singles.tile([128, CO], F32)
            nc.sync.dma_start(
                out=biasb,
                in_=bass.AP(tensor=bias_d, offset=0, ap=[[0, 128], [1, CO]]),
            )
            sel_sb = singles.tile([128, RPG], F32)
            nc.sync.dma_start(out=sel_sb, in_=sel_d[:, :])
            selT_sb = singles.tile([RPG, 128], F32)
            nc.sync.dma_start(out=selT_sb, in_=selT_d[:, :])

            eps_sb = singles.tile([128, 1], F32)
            nc.vector.memset(eps_sb, 1e-8)
            junk = singles.tile([1, 1], F32)
            nc.vector.memset(junk, 1.0)
            # Prefetch the Sqrt activation table while inputs load.
            tl0 = singles.tile([1, 1], F32, tag="tl0")
            nc.scalar.activation(tl0, junk, ACT.Sqrt)

            esr = None   # [16, 99]: exp(b_ij) ++ 1/sum_c exp(b_ij)

            for it in range(ITERS):
                if it == 0:
                    # ---- s0 = 0.1*(x~full @ W~full): c is uniform, so
                    # every core computes the full [256,160] locally.
                    s_ps = [psum_s.tile([128, CO], F32, tag=f"s{kb}",
                                        name=f"s_ps{kb}_0")
                            for kb in range(NB)]
                    for cc in range(N_CORES):
                        for g in range(NG):
                            for kb in range(NB):
                                nc.tensor.matmul(
                                    s_ps[kb],
                                    XTF[cc][:, g, kb * 128:(kb + 1) * 128],
                                    WGF[cc][:, g, :],
                                    start=(cc == 0 and g == 0),
                                    stop=(cc == N_CORES - 1 and g == NG - 1),
                                )
                    t = work.tile([128, NB, CO], F32, tag="t", name="t_0")
                    for kb in range(NB):
                        nc.vector.scalar_tensor_tensor(
                            out=t[:, kb, :], in0=s_ps[kb], scalar=0.1,
                            in1=biasb, op0=ALU.mult, op1=ALU.add,
                        )
                else:
                    # ---- CW = c∘W~ from esr of the previous iteration ----
                    cp_ps = psum_misc.tile([128, NG * C + NG], F32, tag="cp",
                                           name=f"cp_ps_{it}")
                    nc.tensor.matmul(cp_ps, selT_sb, esr, start=True, stop=True)
                    cp_sb = small.tile([128, NG * C + NG], F32, tag="cpart",
                                       name=f"cp_sb_{it}")
                    nc.vector.tensor_copy(cp_sb, cp_ps)
                    CW = cw_pool.tile([128, NG, CO], F16, tag="cw",
                                      name=f"cw_{it}")
                    NGP = 6          # groups on DVE via stt; rest on gpsimd
                    cn = small.tile([128, (NG - NGP) * C], F32, tag="cn",
                                    name=f"cn_{it}")
                    rec_b3 = cp_sb[:, NG * C + NGP:NG * C + NG].rearrange(
                        "p (g one) -> p g one", one=1
                    ).broadcast_to([128, NG - NGP, C])
                    nc.vector.tensor_tensor(
                        out=cn.rearrange("p (g c) -> p g c", g=NG - NGP),
                        in0=cp_sb[:, NGP * C:NG * C].rearrange(
                            "p (g c) -> p g c", g=NG - NGP),
                        in1=rec_b3, op=ALU.mult,
                    )
                    for g in range(NG):
                        if g < NGP:
                            e_b = cp_sb[:, g * C:(g + 1) * C].rearrange(
                                "p (c one) -> p c one", one=1
                            ).broadcast_to([128, C, O])
                            nc.vector.scalar_tensor_tensor(
                                out=CW[:, g, :].rearrange(
                                    "p (c o) -> p c o", c=C),
                                in0=WG[:, g, :].rearrange(
                                    "p (c o) -> p c o", c=C),
                                scalar=cp_sb[:, NG * C + g:NG * C + g + 1],
                                in1=e_b,
                                op0=ALU.mult, op1=ALU.mult,
                            )
                        else:
                            c_b = cn[:, (g - NGP) * C:(g - NGP + 1) * C
                                     ].rearrange(
                                "p (c one) -> p c one", one=1
                            ).broadcast_to([128, C, O])
                            nc.gpsimd.tensor_tensor(
                                out=CW[:, g, :].rearrange(
                                    "p (c o) -> p c o", c=C),
                                in0=WG[:, g, :].rearrange(
                                    "p (c o) -> p c o", c=C),
                                in1=c_b, op=ALU.mult,
                            )

                    # ---- s partial: [256,160] = x~^T @ CW, K = (ri) ----
                    s_ps = [psum_s.tile([128, CO], F32, tag=f"s{kb}",
                                        name=f"s_ps{kb}_{it}")
                            for kb in range(NB)]
                    for kb in range(NB):
                        for g in range(NG):
                            nc.tensor.matmul(
                                s_ps[kb],
                                XT[:, g, kb * 128:(kb + 1) * 128],
                                CW[:, g, :],
                                start=(g == 0),
                                stop=(g == NG - 1),
                            )

                    # Partition-major collective buffer: row p holds s for
                    # batches (p, 128+p) at columns [0:CO] / [CO:2*CO].
                    cc_in = dram.tile([128, NB * CO], F16, tag="cc_in",
                                      name=f"cc_in_{it}")
                    for kb in range(NB):
                        s_stage = work.tile([128, CO], F16, tag=f"sstage{kb}",
                                            name=f"s_stage{kb}_{it}")
                        nc.scalar.copy(s_stage, s_ps[kb])
                        nc.sync.dma_start(
                            out=cc_in[:, kb * CO:(kb + 1) * CO], in_=s_stage
                        )

                    if it < ITERS - 1:
                        # ---- AllReduce s (doubles as the rank barrier) ----
                        cc_out = dram.tile([128, NB * CO], F16, tag="cc_out",
                                           name=f"cc_out_{it}")
                        nc.gpsimd.collective_compute(
                            "AllReduce", ALU.add, replica_groups=rg,
                            ins=[cc_in.opt()], outs=[cc_out.opt()],
                        )
                        s_sb = work.tile([128, NB, CO], F16, tag="ssb",
                                         name=f"s_sb_{it}")
                        nc.sync.dma_start(
                            out=s_sb.rearrange("p nb co -> p (nb co)"),
                            in_=cc_out[:, :],
                        )
                        # Keep the PE HAM busy during the AllReduce so the
                        # G matmuls start at full clock.
                        warm_ps = psum_misc.tile([128, 512], F32,
                                                 tag="warmps",
                                                 name=f"warm_ps_{it}")
                        warm_rhs = XT[:, 0, :]        # [128, 256] static
                        warm_lhs = XT[:, 0, :128]     # [128, 128] fp16
                        for wi in range(WARM_MM):
                            nc.tensor.matmul(
                                warm_ps[:, :B], warm_lhs, warm_rhs,
                                start=(wi == 0), stop=True,
                                skip_group_check=True,
                            )
                        t = work.tile([128, NB, CO], F32, tag="t",
                                      name=f"t_{it}")
                        bias_b = biasb.rearrange(
                            "p (one co) -> p one co", one=1
                        ).broadcast_to([128, NB, CO])
                        nc.vector.scalar_tensor_tensor(
                            out=t, in0=s_sb, scalar=1.0,
                            in1=bias_b, op0=ALU.mult, op1=ALU.add,
                        )
                    else:
                        # ---- final iter: ReduceScatter; own shard only ----
                        # Shard k of the flat [128*NB*CO] buffer = partition
                        # rows [16k, 16k+16) = batches 16k+j and 128+16k+j.
                        rs_out = dram.tile([P_SHARD * NB * CO], F16,
                                           tag="rs_out")
                        nc.gpsimd.collective_compute(
                            "ReduceScatter", ALU.add, replica_groups=rg,
                            ins=[cc_in.opt()], outs=[rs_out[:]],
                        )
                        s_sb = work.tile([P_SHARD, NB, CO], F16, tag="fs")
                        nc.sync.dma_start(
                            out=s_sb,
                            in_=rs_out.rearrange("(p nb n) -> p nb n",
                                                 n=CO, nb=NB),
                        )
                        t = work.tile([P_SHARD, NB, CO], F32, tag="ft")
                        bias_b1 = biasb[:P_SHARD, :].rearrange(
                            "p (one co) -> p one co", one=1
                        ).broadcast_to([P_SHARD, NB, CO])
                        nc.vector.scalar_tensor_tensor(
                            out=t, in0=s_sb, scalar=1.0,
                            in1=bias_b1, op0=ALU.mult, op1=ALU.add,
                        )
                        v = _squash(nc, eps_sb, t, P_SHARD, NB, work,
                                    out_dt=F32)
                        nc.sync.dma_start(
                            out=y_d[:, :],
                            in_=v.rearrange("p nb co -> p (nb co)"),
                        )
                        break

                v_sb = _squash(nc, eps_sb, t, 128, NB, work, out_dt=F16)
                # Prefetch the Exp table (runs during the G matmuls).
                tlE = small.tile([1, 1], F32, tag="tlE", name=f"tlE_{it}")
                nc.scalar.activation(tlE, junk, ACT.Exp)

                # ---- G = (1/B) x~^T v ; agree = sum_io W∘G ----
                Q_all = small.tile([128, NG * C], F32, tag="qall",
                                   name=f"qall_{it}")
                p9 = work.tile([128, NG, CO], F32, tag="p9",
                               name=f"p9_{it}")
                for g in range(NG):
                    g_ps = psum_g.tile([128, CO], F32, tag="gps",
                                       name=f"g_ps_{it}_{g}")
                    for kb in range(NB):
                        nc.tensor.matmul(
                            g_ps,
                            XB[kb][:, g * 128:(g + 1) * 128],
                            v_sb[:, kb, :],
                            start=(kb == 0),
                            stop=(kb == NB - 1),
                        )
                    nc.vector.scalar_tensor_tensor(
                        out=p9[:, g, :], in0=g_ps, scalar=1.0 / B,
                        in1=WG[:, g, :], op0=ALU.mult, op1=ALU.mult,
                    )
                    if g == 3 or g == 7 or g == 8:
                        lo = 0 if g == 3 else (4 if g == 7 else 8)
                        nc.vector.reduce_sum(
                            Q_all[:, lo * C:(g + 1) * C],
                            p9[:, lo:g + 1, :].rearrange(
                                "p g (c o) -> p (g c) o", c=C),
                            axis=mybir.AxisListType.X,
                        )
                agree_ps = psum_misc.tile([RPG, NG * C], F32, tag="agree",
                                          name=f"agree_{it}")
                nc.tensor.matmul(agree_ps, sel_sb, Q_all,
                                 start=True, stop=True)

                # ---- exp(b_ij) updated multiplicatively:
                # exp(b_prev + agree) = exp(b_prev) * exp(agree) ----
                esr_prev = esr
                esr = small.tile([RPG, NG * C + NG], F32, tag="esr",
                                 name=f"esr_{it}")
                if it == 0:
                    nc.scalar.activation(esr[:, :NG * C], agree_ps, ACT.Exp)
                else:
                    eexp = small.tile([RPG, NG * C], F32, tag="eexp",
                                      name=f"eexp_{it}")
                    nc.scalar.activation(eexp, agree_ps, ACT.Exp)
                    nc.vector.tensor_mul(
                        esr[:, :NG * C], esr_prev[:, :NG * C], eexp
                    )
                # Prefetch Sqrt for the next squash (runs during CW/s).
                tlS = small.tile([1, 1], F32, tag="tlS", name=f"tlS_{it}")
                nc.scalar.activation(tlS, junk, ACT.Sqrt)
                den = small.tile([RPG, NG], F32, tag="sden",
                                 name=f"den_{it}")
                nc.vector.reduce_sum(
                    den,
                    esr[:, :NG * C].rearrange("p (g c) -> p g c", g=NG),
                    axis=mybir.AxisListType.X,
                )
                nc.vector.reciprocal(esr[:, NG * C:], den)

    nc.compile()
    return nc


_NC = None


def kernel(x: np.ndarray, W: np.ndarray, bias: np.ndarray) -> np.ndarray:
    global _NC
    if _NC is None:
        _NC = build()

    x = np.ascontiguousarray(x, dtype=np.float32)
    W = np.ascontiguousarray(W, dtype=np.float32)
    bias = np.ascontiguousarray(bias, dtype=np.float32)

    biasf = bias.reshape(CO)
    sel = np.zeros((128, RPG), dtype=np.float32)
    sel[np.arange(128), np.arange(128) // I] = 1.0
    selT = np.ascontiguousarray(sel.T)

    xf = x.reshape(B, R * I)                              # [B,(r,i)]
    xtf = np.ascontiguousarray(xf.T).astype(np.float16)   # [(r,i),B]
    wgf = np.ascontiguousarray(
        W.transpose(0, 3, 1, 2).reshape(R * I, CO)).astype(np.float16)

    in_maps = []
    for k in range(N_CORES):
        r0, r1 = k * R_LOC, (k + 1) * R_LOC
        xk = x[:, r0:r1, :].reshape(B, RI_LOC)          # [B,(r,i)]
        wk = W[r0:r1].transpose(0, 3, 1, 2).reshape(RI_LOC, CO)  # [(ri),(co)]
        in_maps.append({
            "xtf": xtf,
            "wgf": wgf,
            "xt": np.ascontiguousarray(xk.T).astype(np.float16),
            "xb": np.ascontiguousarray(xk).astype(np.float16),
            "wg": np.ascontiguousarray(wk).astype(np.float16),
            "biasf": biasf,
            "sel": sel,
            "selT": selT,
        })

    global LAST_RESULT
    res = run_bass_kernel_spmd(
        _NC, in_maps, list(range(N_CORES)),
        trace=bool(os.environ.get("BASS_TRACE")),
    )
    LAST_RESULT = res
    # Reassemble: core k, row j, chunk kb  ->  batch kb*128 + 16*k + j.
    out = np.empty((B, CO), dtype=np.float32)
    for k in range(N_CORES):
        yk = res.results[k]["y"].reshape(P_SHARD, NB, CO)
        for kb in range(NB):
            out[kb * 128 + P_SHARD * k: kb * 128 + P_SHARD * (k + 1)] = \
                yk[:, kb, :]
    return out.reshape(B, C, O)[..., None].astype(np.float32)
